# revision 21
# baseline (speedup 1.0000x reference)
"""Trainium2 Bass kernel for nn_ExampleEncoderLayer (dense transformer block).

Sharding: hybrid batch x sequence over 8 cores = 4 batches x 2 L-halves.
Per core (batch n, half): BN(x) -> h0 (full L, for K/V); Q + attention for
its 512-column window (inputs pre-rolled on host so the window is always
local columns [0,512)); out-projection + residual. The IbnNet conv stack
then switches to channel sharding: conv1 stays L-local (all mids), a pair
AllGather reassembles y1 over the full global L, conv2 computes this
core's 512 mid channels (weight columns sliced host-side) over full L,
a second AllGather rebuilds y2 in global mid order, and conv3 computes
this core's 512 OUTPUT channels (channel space rolled per core host-side
so the SPMD program is uniform) over full L -- which makes the
instance-norm statistics core-local: no stats AllReduce, no conv2 halo
exchange. The attention residual h is pair-exchanged (AllGather + mask
blend) to cover the remote L-half. All collectives are chunked so they
overlap the neighboring matmul phases.

v2: weights/activations in bf16 (same PE rate as f32r, half the HBM/SBUF
traffic); K/Q/V/exp attention operands in fp8e4 (raw exp(s) is O(1) so the
range fits; the whole attention branch contributes ~1.3% of the residual
so fp8's ~4% relative noise lands ~1e-4 on the output, far under the 2e-2
budget). The kernel front is software-pipelined per attention PAIR: the
softmax exp stream on the Activation engine (~75us, the real bottleneck of
the attention phase) starts ~17us in and hides under the K/Q/V/AV matmuls
instead of serializing after them. GpSimd drains the K/Q/V psums so the
DVE queue stays on the oT/den/residual path. The softmax 1/sqrt(d_model)
is applied as the exp ACTIVATE's scale constant so q/k stay at full scale
for fp8.
"""

import sys
import os

for _p in ("/opt/trn_rl_repo", "/root/.axon_site/_ro/trn_rl_repo"):
    if os.path.isdir(_p) and _p not in sys.path:
        sys.path.insert(0, _p)

import numpy as np
import ml_dtypes

E4 = ml_dtypes.float8_e4m3fn

import concourse.tile as tile
from concourse import bacc, mybir
from concourse import bass_utils

F32 = mybir.dt.float32
F32R = mybir.dt.float32r
BF16 = mybir.dt.bfloat16
FP8 = mybir.dt.float8e4
AF = mybir.ActivationFunctionType
ALU = mybir.AluOpType
AX = mybir.AxisListType
DR = mybir.MatmulPerfMode.DoubleRow

C = 1024      # d_model / channels / mid_channels
L = 1024      # sequence length
N_BATCH = 4
W = 512       # per-core L window
NT = C // 128  # 8 channel tiles
HEADS = 16
DH = 64
PAIRS = 8     # head pairs (2 heads = 128 partitions)
EPS = 1e-5
RG = [[0, 1], [2, 3], [4, 5], [6, 7]]  # core pairs sharing a batch

TRACE = False
LAST_RESULTS = None


def _build():
    from contextlib import ExitStack

    nc = bacc.Bacc("TRN2", target_bir_lowering=False, debug=False, num_devices=8)

    x_d = nc.dram_tensor("x", [C, L], BF16, kind="ExternalInput").ap()
    wqT_d = nc.dram_tensor("wqT", [C, C], FP8, kind="ExternalInput").ap()
    wkT_d = nc.dram_tensor("wkT", [C, C], FP8, kind="ExternalInput").ap()
    wvT_d = nc.dram_tensor("wvT", [C, C], FP8, kind="ExternalInput").ap()
    woT_d = nc.dram_tensor("woT", [C, C], FP8, kind="ExternalInput").ap()
    l1T_d = nc.dram_tensor("l1T", [C, C], BF16, kind="ExternalInput").ap()
    l2T_d = nc.dram_tensor("l2T", [3, C, C], FP8, kind="ExternalInput").ap()
    # conv3 weights hold only this core's 512 output channels
    l3T_d = nc.dram_tensor("l3T", [C, C // 2], BF16, kind="ExternalInput").ap()
    # packed per-channel columns: s0 t0 b1 b2 (8 each) b3 (4) mA mB cinv
    vecs_d = nc.dram_tensor("vecs", [128, 39], F32, kind="ExternalInput").ap()
    # 2x128 selector for the denominator broadcast matmul:
    # row 0 = [1]*64+[0]*64, row 1 = [0]*64+[1]*64
    selm_d = nc.dram_tensor("selm", [2, 128], F32R, kind="ExternalInput").ap()
    # this core's 512 (rolled) channels x full pooled length
    out_d = nc.dram_tensor("out", [C // 2, L // 2], F32, kind="ExternalOutput").ap()

    with tile.TileContext(nc) as tc:
      with (
        tc.tile_pool(name="pmisc", bufs=1) as pm,
        tc.tile_pool(name="pB", bufs=1) as pB,
        tc.tile_pool(name="dram", bufs=1, space="DRAM") as dp,
      ):
        vecs = pm.tile([128, 39], F32, tag="vecs")
        nc.scalar.dma_start(out=vecs[:], in_=vecs_d)
        s0 = vecs[:, 0:8]
        t0 = vecs[:, 8:16]
        b1 = vecs[:, 16:24]
        b2 = vecs[:, 24:32]
        b3 = vecs[:, 32:36]
        mA = vecs[:, 36:37]
        mB = vecs[:, 37:38]
        cinv = vecs[:, 38:39]

        def wdma(**kw):
            # all weight streams on the sync HWDGE queue: scalar is reserved
            # for ACT(exp) + x staging, gpsimd for psum drains + collectives
            nc.sync.dma_start(**kw)

        ones_f = pm.tile([128, 2], F32, tag="ones_f")
        nc.vector.memset(ones_f[:], 1.0)
        selm = pm.tile([2, 128], F32R, tag="selm")
        nc.sync.dma_start(out=selm[:], in_=selm_d)

        # conv-phase buffers (persist past the attention pool)
        h = [pB.tile([128, W], BF16, tag=f"h{i}", name=f"h{i}")
             for i in range(NT)]
        c1 = pB.tile([128, NT, C], BF16, tag="c1band")

        stA = ExitStack()
        pA = stA.enter_context(tc.tile_pool(name="pA", bufs=1))

        # h0 split: window half (lives through the residual) and far half
        # (only needed for K/V)
        h0a = pA.tile([128, NT, W], BF16, tag="h0a")
        h8a = pA.tile([128, NT, W], FP8, tag="h8a")
        h8b = pA.tile([128, NT, L - W], FP8, tag="h8b")
        v_sb = pA.tile([128, NT, HEADS, DH + 1], FP8, tag="v_sb")
        nc.vector.tensor_copy(
            out=v_sb[:, :, :, DH:DH + 1],
            in_=ones_f[:, 0:1].broadcast_to((128, NT * HEADS)).rearrange(
                "p (a h) -> p a h", a=NT).unsqueeze(3))
        kT = [pA.tile([128, L], FP8, tag=f"kT{i}", name=f"kT{i}")
              for i in range(PAIRS)]
        # Q^T raw pair layout: head-A dims on partitions 0:64, head-B on
        # 64:128. Scores run as K=64 row-TILED matmul pairs (tile_position
        # (0,0)/(64,0) auto-derived from the slices) so both heads' score
        # tiles stream concurrently through the PE array.
        qT = [pA.tile([128, W], FP8, tag=f"qT{i}", name=f"qT{i}")
              for i in range(PAIRS)]
        oT = [pA.tile([128, W], BF16, tag=f"oT{i}", name=f"oT{i}")
              for i in range(PAIRS)]
        o8 = pA.tile([128, PAIRS, W], FP8, tag="o8")

        def h8key(ct, khalf):
            # key-half view of BN(x), fp8: 0 -> window half, 1 -> far half
            return h8a[:, ct, :] if khalf == 0 else h8b[:, ct, :]

        def h8pair(a, khalf, kcols=None):
            t = h8a if khalf == 0 else h8b
            v = t[:, 2 * a:2 * a + 2, :]
            return v if kcols is None else v[:, :, kcols[0]:kcols[1]]

        # --- attention bookkeeping shared by the emission helpers ---
        # per-PAIR exp tiles [128, head, kt, W] so one exp ACT covers both
        # heads of a key tile and the AV DoubleRow rhs [128, 2, W] slices out
        expT = [pA.tile([128, 2, NT, W], FP8, tag=f"expT{i}",
                        name=f"expT{i}") for i in range(PAIRS)]
        den2s = [None] * PAIRS
        dden = dp.tile([HEADS, W], F32, tag="dden")

        # PSUM pools, LIFO-ordered. Budget 8 banks of 2KB/partition:
        #   spsq (scores, 2x2 banks double-buffered) resident through
        #   attention + psA (K/Q, 3) during the K/Q phase, psV (3) during V,
        #   psO (AV, 2) + dpsn (den bcast, 1) during AV; all closed before
        #   the out-projection opens psW (4).
        stS = ExitStack()
        spsq = stS.enter_context(tc.tile_pool(name="sc_ps", bufs=2, space="PSUM"))
        psO = None  # AV psum pool: opened after the merged K/Q/V phase

        # ---------------- emission helpers ----------------
        sunits = []   # pending (pr, kt) score+exp units

        def stage_scores(pr):
            for kt in range(NT):
                sunits.append((pr, kt))

        def emit_sunit():
            if not sunits:
                return
            pr, kt = sunits.pop(0)
            sq = spsq.tile([128, 2, W], F32, tag="sq", name="sq")
            # row-tiled K=64 pair: head A in array rows 0:63, head B in
            # 64:127, streaming concurrently (tile_position auto-derives
            # from the slices' base partitions)
            nc.tensor.matmul(sq[:, 0, :], kT[pr][0:DH, kt * 128:(kt + 1) * 128],
                             qT[pr][0:DH, :])
            nc.tensor.matmul(sq[:, 1, :], kT[pr][DH:128, kt * 128:(kt + 1) * 128],
                             qT[pr][DH:128, :])
            # one ACT call per 2 banks (the 352-cycle ACTIVATE overhead is
            # per instruction); the softmax /sqrt(d_model) rides the free
            # affine scale
            nc.scalar.activation(out=expT[pr][:, :, kt, :],
                                 in_=sq[:], func=AF.Exp, scale=1.0 / 32768.0)

        def emit_sunits(n):
            for _ in range(n):
                emit_sunit()

        def emit_av(head):
            # AV for one head (DoubleRow: two key tiles per matmul); stash
            # UNNORMALIZED o^T; denominator row (the ones-column of v_sb)
            # goes to partitions 0/1 of den2f via a DRAM bounce (a partition
            # move the DVE cannot do)
            pr, hh = divmod(head, 2)
            ops = psO.tile([DH + 1, W], F32, tag="po", name="avps")
            for g in range(NT // 2):
                nc.tensor.matmul(
                    ops[:], v_sb[:, 2 * g:2 * g + 2, head, :],
                    expT[pr][:, hh, 2 * g:2 * g + 2, :],
                    start=(g == 0), stop=(g == NT // 2 - 1),
                    perf_mode=DR)
            lo, hi = hh * DH, (hh + 1) * DH
            nc.vector.tensor_copy(out=oT[pr][lo:hi, :], in_=ops[0:DH, :])
            denst = pm.tile([128, W], F32, tag="denst", bufs=2)
            nc.vector.tensor_copy(out=denst[DH:DH + 1, :],
                                  in_=ops[DH:DH + 1, :])
            nc.gpsimd.dma_start(out=dden[head:head + 1, :],
                                in_=denst[DH:DH + 1, :])
            if hh == 1:
                den2f = pm.tile([2, W], F32, tag="den2f", bufs=2)
                nc.gpsimd.dma_start(out=den2f[:],
                                    in_=dden[2 * pr:2 * pr + 2, :])
                den2r = pm.tile([2, W], F32, tag="den2r", bufs=2)
                nc.vector.reciprocal_approx_fast(out=den2r[:], in_=den2f[:])
                den2 = pm.tile([2, W], F32R, tag="den2", bufs=3)
                nc.vector.tensor_copy(out=den2[:], in_=den2r[:])
                den2s[pr] = den2

        avq = list(range(HEADS))  # heads whose AV is still pending

        def emit_avs(n):
            for _ in range(n):
                if avq:
                    emit_av(avq.pop(0))

        # ---------------- BN + per-pair K/Q, pipelined -------------------
        with tc.tile_pool(name="wband", bufs=4) as wb, \
             tc.tile_pool(name="wbandv", bufs=2) as wbv, \
             tc.tile_pool(name="xstage", bufs=3) as xsp:
          with tc.tile_pool(name="kq_ps", bufs=1, space="PSUM") as psA:
            # warm the PE clock (HAM) with throwaway matmuls while the x/
            # weight DMAs are in flight; ~3.4us of PE activity flips the
            # clock gate to 8/8 before the real work arrives
            wps = psA.tile([128, 2, W], F32, tag="kq", bufs=1)
            for i in range(36):
                nc.tensor.matmul(wps[:, 0, 0:128], selm[:], selm[:, 0:128],
                                 start=True, stop=True)

            # resident wk/wq; low halves first so pair 0 starts ASAP,
            # x tiles next, high halves after (contiguous half-DMAs hit
            # HBM line rate)
            kqK = wb.tile([128, NT // 2, 2, C], FP8, tag="kqK", bufs=1)
            kqQ = wb.tile([128, NT // 2, 2, C], FP8, tag="kqQ", bufs=1)
            x_sbs = []

            def stage_x(ct):
                x_sb = xsp.tile([128, L], BF16, tag="xs", name=f"xs{ct}")
                nc.sync.dma_start(out=x_sb[:],
                                  in_=x_d[ct * 128:(ct + 1) * 128, :])
                x_sbs.append(x_sb)

            # first two x tiles ahead of the weights: the BN chain that
            # gates the first kT drain (and so the whole exp stream) starts
            # as early as possible
            stage_x(0)
            stage_x(1)
            wdma(out=kqK[:, :, :, 0:512],
                 in_=wkT_d[:, 0:512].rearrange(
                     "(a two p) c -> p a two c", two=2, p=128))
            wdma(out=kqQ[:, :, :, 0:512],
                 in_=wqT_d[:, 0:512].rearrange(
                     "(a two p) c -> p a two c", two=2, p=128))
            for ct in range(2, NT):
                stage_x(ct)
            wdma(out=kqK[:, :, :, 512:1024],
                 in_=wkT_d[:, 512:1024].rearrange(
                     "(a two p) c -> p a two c", two=2, p=128))
            wdma(out=kqQ[:, :, :, 512:1024],
                 in_=wqT_d[:, 512:1024].rearrange(
                     "(a two p) c -> p a two c", two=2, p=128))
            # V weight bands prefetched now; the V block runs right after
            # the K/Q pairs
            vbs = []
            for g in range(2):
                vb = wbv.tile([128, NT // 2, 2, W], FP8, tag="vband",
                              name=f"vb{g}", bufs=2)
                wdma(out=vb[:],
                     in_=wvT_d[:, g * 512:(g + 1) * 512].rearrange(
                         "(a two p) c -> p a two c", two=2, p=128))
                vbs.append(vb)

            for pr in range(PAIRS):
                kps = psA.tile([128, 2, W], F32, tag="kq", bufs=1)
                qps = psA.tile([128, W], F32, tag="q", bufs=1)
                for a in range(NT // 2):
                    if pr == 0:
                        # BN as the x tiles land (first pair only)
                        for ct in (2 * a, 2 * a + 1):
                            nc.vector.tensor_scalar(
                                out=h0a[:, ct, :], in0=x_sbs[ct][:, 0:W],
                                scalar1=s0[:, ct:ct + 1],
                                scalar2=t0[:, ct:ct + 1],
                                op0=ALU.mult, op1=ALU.add)
                            nc.gpsimd.tensor_copy(out=h8a[:, ct, :],
                                                   in_=h0a[:, ct, :])
                            nc.gpsimd.tensor_scalar(
                                out=h8b[:, ct, :], in0=x_sbs[ct][:, W:L],
                                scalar1=s0[:, ct:ct + 1],
                                scalar2=t0[:, ct:ct + 1],
                                op0=ALU.mult, op1=ALU.add)
                    for kh in range(2):
                        nc.tensor.matmul(
                            kps[:, kh, :],
                            kqK[:, a, :, pr * 128:(pr + 1) * 128],
                            h8pair(a, kh),
                            start=(a == 0), stop=(a == NT // 2 - 1),
                            perf_mode=DR)
                    nc.tensor.matmul(
                        qps[:], kqQ[:, a, :, pr * 128:(pr + 1) * 128],
                        h8pair(a, 0),
                        start=(a == 0), stop=(a == NT // 2 - 1),
                        perf_mode=DR)
                    # two score units of the previous pair between K/Q
                    # steps keep the exp stream fed from ~one pair in
                    if pr >= 1:
                        emit_sunits(2)
                # drains on gpsimd so the DVE stays free for the oT path;
                # the PE chews queued score units while they run
                nc.vector.tensor_copy(
                    out=kT[pr][:].rearrange("p (a w) -> p a w", a=2),
                    in_=kps[:])
                nc.vector.tensor_copy(out=qT[pr][:], in_=qps[:])
                stage_scores(pr)

          # ---------------- V projection, dense block ----------------
          # psA closed: its 3 banks host the V psums; pair 7's score units
          # interleave so the exp stream never starves while V streams.
          with tc.tile_pool(name="v_ps", bufs=1, space="PSUM") as psV:
            for g in range(2):
                for ci, chunk in enumerate(((0, 1, 2), (3, 4, 5), (6, 7))):
                    vps = psV.tile([128, 3, W], F32, tag="vps",
                                   name=f"vps{g}{ci}")
                    for a in range(NT // 2):
                        for i, kt in enumerate(chunk):
                            kh, kcol = divmod(kt * 128, W)
                            nc.tensor.matmul(
                                vps[:, i, :],
                                h8pair(a, kh, (kcol, kcol + 128)),
                                vbs[g][:, a, :, :],
                                start=(a == 0), stop=(a == NT // 2 - 1),
                                perf_mode=DR)
                    for i, kt in enumerate(chunk):
                        nc.vector.tensor_copy(
                            out=v_sb[:, kt, g * 8:(g + 1) * 8, 0:DH],
                            in_=vps[:, i, :].rearrange(
                                "p (h d) -> p h d", h=8))

        # throwaway pair-AllReduce: synchronizes the cores early so the
        # conv-phase AllGathers do not pay the accumulated trigger skew
        cc0i = dp.tile([128, 1], F32, tag="cc0i")
        cc0o = dp.tile([128, 1], F32, tag="cc0o")
        nc.sync.dma_start(out=cc0i[:], in_=ones_f[:, 0:1])
        nc.gpsimd.collective_compute(
            "AllReduce", ALU.add, replica_groups=RG,
            ins=[cc0i[:].opt()], outs=[cc0o[:].opt()])

        # outproj wo bands (fp8 pair layout), prefetched now
        obs = []
        for gi in range(2):
            ob = pA.tile([128, NT // 2, 2, W], FP8,
                         tag=f"oband{gi}", name=f"oband{gi}")
            wdma(out=ob[:],
                 in_=woT_d[:, gi * 512:(gi + 1) * 512].rearrange(
                     "(a two p) c -> p a two c", two=2, p=128))
            obs.append(ob)

        # ---------------- AV (DoubleRow) + normalization ----------------
        from contextlib import ExitStack as _ES
        stP = _ES()
        psO = stP.enter_context(tc.tile_pool(name="av_ps", bufs=2,
                                             space="PSUM"))
        dpsn = stP.enter_context(tc.tile_pool(name="dn_ps", bufs=1,
                                              space="PSUM"))

        def emit_norm(p):
            # broadcast both heads' 1/den with one K=2 matmul, then
            # scale o^T in place
            dps = dpsn.tile([128, W], F32, tag="dn", name="dnps")
            nc.tensor.matmul(dps[:], selm[:], den2s[p][:])
            nc.vector.tensor_mul(out=o8[:, p, :], in0=oT[p][:],
                                 in1=dps[:])

        # prefetch conv1 weights before the AV/outproj phase (the sync
        # queue is otherwise idle here and conv1 starts right after)
        wdma(out=c1[:], in_=l1T_d[:].rearrange("(a p) c -> p a c", p=128))

        for p in range(PAIRS):
            emit_avs(2)
            # norm of the pair whose den-reciprocal chain (DRAM bounce +
            # DVE) has certainly completed; lag 2 pairs
            if p >= 2:
                emit_norm(p - 2)
            if p == 5:
                # pair 7's score units, paced by the exp stream's sq-bank
                # releases (the stream has nearly drained by now)
                emit_sunits(8)
        emit_norm(PAIRS - 2)
        emit_norm(PAIRS - 1)

        # tiny skew-absorber gated on late-attention data: it soaks up the
        # cross-core jitter accumulated over the attention phase so the
        # conv-phase exchanges below process at their ~5us floor
        cc2i = dp.tile([128, 1], BF16, tag="cc2i")
        cc2o = dp.tile([2, 128, 1], BF16, tag="cc2o")
        nc.sync.dma_start(out=cc2i[:], in_=oT[7][:, 0:1])
        nc.gpsimd.collective_compute(
            "AllGather", ALU.bypass, replica_groups=RG,
            ins=[cc2i[:].opt()], outs=[cc2o[:].opt()])
        stP.close()  # AV + den psum pools close
        stS.close()  # scores psum pool closes

        # second throwaway pair-sync: absorbs the core skew accumulated
        # across the attention phase so the conv-phase AllGathers process
        # at their ~5us floor instead of paying the slowest core's lag
        cc1i = dp.tile([128, 1], F32, tag="cc1i")
        cc1o = dp.tile([128, 1], F32, tag="cc1o")
        nc.sync.dma_start(out=cc1i[:], in_=ones_f[:, 0:1])
        nc.gpsimd.collective_compute(
            "AllReduce", ALU.add, replica_groups=RG,
            ins=[cc1i[:].opt()], outs=[cc1o[:].opt()])

        # ---------------- out-projection ----------------
        # h tiles 4:8 first so their pair-exchange staging can start while
        # tiles 0:4 are still accumulating
        hxi = dp.tile([128, 4, W], BF16, tag="hxi")
        hxo = dp.tile([2, 128, 4, W], BF16, tag="hxo")
        with tc.tile_pool(name="wo_ps", bufs=2, space="PSUM") as psW:

            def op_group(cts, kts, pss, first, last):
                gi = cts[0] // 4
                for kt in kts:
                    for i, ct in enumerate(cts):
                        nc.tensor.matmul(
                            pss[i][:],
                            obs[gi][:, kt // 2, kt % 2,
                                    (ct % 4) * 128:(ct % 4 + 1) * 128],
                            o8[:, kt, :],
                            start=(kt == first), stop=(kt == last))

            for gX in ((4, 5), (6, 7), (0, 1), (2, 3)):
                wopX = [psW.tile([128, W], F32, tag="wo",
                                 name=f"wop{gX[0]}_{i}") for i in range(2)]
                op_group(gX, tuple(range(NT)), wopX, 0, NT - 1)
                for i, ct in enumerate(gX):
                    nc.vector.scalar_tensor_tensor(
                        out=h[ct][:], in0=wopX[i][:], scalar=cinv,
                        in1=h0a[:, ct, :], op0=ALU.mult, op1=ALU.add)
                if gX == (6, 7):
                    for i in range(4):
                        nc.gpsimd.dma_start(out=hxi[:, i, :],
                                            in_=h[4 + i][:])

        # attention-phase SBUF is no longer needed; conv buffers take its
        # place in pools opened only now (pools close LIFO, hence the split).
        stA.close()
        stB = ExitStack()
        pC = stB.enter_context(tc.tile_pool(name="pC", bufs=1))
        h_full = pC.tile([128, 4, L], BF16, tag="h_full")
        with (
            tc.tile_pool(name="c1_ps", bufs=8, space="PSUM") as ps1,
        ):
            # resident conv2/conv3 weights, streamed during conv1
            c2b = pC.tile([128, 3, 4, 2, C], FP8, tag="c2b")
            for tap in range(3):
                nc.sync.dma_start(
                    out=c2b[:, tap, :, :, :],
                    in_=l2T_d[tap].rearrange("(a two p) c -> p a two c",
                                             two=2, p=128))
            c3 = pC.tile([128, NT, W], BF16, tag="c3b")
            nc.sync.dma_start(
                out=c3[:], in_=l3T_d[:].rearrange("(a p) c -> p a c", p=128))

            # ---------------- conv1 (1x1) + bn1 + relu ----------------
            # L-local: all 1024 mids x my 512 cols. y1 col j+1 = my col j;
            # cols 0/513 are the cross-core halo columns (row stride 528
            # keeps the DoubleRow Ko-step 16-aligned).
            y1 = pC.tile([128, NT, 528], FP8, tag="y1")
            c1bands = [c1[:, kt, :] for kt in range(NT)]
            # boundary pre-chain first: the two window-edge output columns
            # feed the tiny edge AllGather that replaces the halo exchange
            bps = [ps1.tile([128, 2], F32, tag="ps", name=f"bps{i}")
                   for i in range(NT)]
            for kt in range(NT):
                for mt in range(NT):
                    nc.tensor.matmul(
                        bps[mt][:], c1bands[kt][:, mt * 128:(mt + 1) * 128],
                        h[kt][:, 0:W:W - 1],
                        start=(kt == 0), stop=(kt == NT - 1))
            bc = pm.tile([128, NT, 2], F32, tag="bc")
            for mt in range(NT):
                nc.vector.tensor_scalar(
                    out=bc[:, mt, :], in0=bps[mt][:],
                    scalar1=b1[:, mt:mt + 1], scalar2=0.0,
                    op0=ALU.add, op1=ALU.max)
            exi = dp.tile([128, 16], F32, tag="exi")
            exo = dp.tile([2, 128, 16], F32, tag="exo")
            nc.gpsimd.dma_start(out=exi[:],
                                in_=bc[:].rearrange("p a b -> p (a b)"))
            # edge exchange first (urgent: conv2 needs it), then the h
            # exchange (slack until the conv3 residual)
            nc.gpsimd.collective_compute(
                "AllGather", ALU.bypass, replica_groups=RG,
                ins=[exi[:].opt()], outs=[exo[:].opt()])
            nc.gpsimd.collective_compute(
                "AllGather", ALU.bypass, replica_groups=RG,
                ins=[hxi[:].opt()], outs=[hxo[:].opt()])
            # edge readback + halo blend: left halo col = peer edge col 511
            # (zero at the global left edge via mB), right halo = peer col 0
            exs = pm.tile([128, 2, NT, 2], F32, tag="exs")
            for s in range(2):
                nc.gpsimd.dma_start(
                    out=exs[:, s, :, :],
                    in_=exo[s].rearrange("p (a b) -> p a b", a=NT))
            nc.vector.tensor_scalar_mul(out=y1[:, :, 0:1],
                                        in0=exs[:, 0, :, 1:2], scalar1=mB)
            nc.vector.tensor_scalar_mul(out=y1[:, :, W + 1:W + 2],
                                        in0=exs[:, 1, :, 0:1], scalar1=mA)

            # h_full blend: remote-L half comes from the peer's hx slot; the
            # mA/mB masks pick the right slot per core (emitted now, runs as
            # soon as the hx AllGather lands; DVE is idle through conv1/2)
            hxs = pC.tile([128, 2, 4, W], BF16, tag="hxs")
            for s in range(2):
                nc.gpsimd.dma_start(out=hxs[:, s, :, :], in_=hxo[s])
            hblt = pC.tile([128, 4, W], BF16, tag="hblt")
            for ct in range(4):
                nc.vector.tensor_scalar_mul(out=hblt[:, ct, :],
                                            in0=hxs[:, 0, ct, :], scalar1=mB)
                nc.vector.scalar_tensor_tensor(
                    out=h_full[:, ct, 0:W], in0=h[ct][:], scalar=mA,
                    in1=hblt[:, ct, :], op0=ALU.mult, op1=ALU.add)
                nc.vector.tensor_scalar_mul(out=hblt[:, ct, :],
                                            in0=hxs[:, 1, ct, :], scalar1=mA)
                nc.vector.scalar_tensor_tensor(
                    out=h_full[:, ct, W:L], in0=h[ct][:], scalar=mB,
                    in1=hblt[:, ct, :], op0=ALU.mult, op1=ALU.add)

            # conv1 main
            pss1 = [ps1.tile([128, W], F32, tag="ps", name=f"c1ps{i}")
                    for i in range(NT)]
            for kt in range(NT):
                for mt in range(NT):
                    nc.tensor.matmul(
                        pss1[mt][:],
                        c1bands[kt][:, mt * 128:(mt + 1) * 128],
                        h[kt][:], start=(kt == 0), stop=(kt == NT - 1))
            for mt in range(NT):
                nc.scalar.activation(out=y1[:, mt, 1:W + 1],
                                     in_=pss1[mt][:], func=AF.Relu,
                                     bias=b1[:, mt:mt + 1], scale=1.0)

        # ---------------- conv2 (k=3) + bn2 + relu, L-local ------------
        # all 1024 mids x my 512 cols; mid-tile groups 0-3 finish first so
        # their y2 AllGather chunk flies while the PE works on 4-7
        with tc.tile_pool(name="c2_ps", bufs=8, space="PSUM") as ps2:
            y2own = pC.tile([128, NT, W], FP8, tag="y2own")
            y2g = pC.tile([128, NT, L], FP8, tag="y2g")
            y2i = [dp.tile([128, 4, W], FP8, tag=f"y2i{g}", name=f"y2i{g}")
                   for g in range(2)]
            y2o = [dp.tile([2, 128, 4, W], FP8, tag=f"y2o{g}", name=f"y2o{g}")
                   for g in range(2)]
            pss2 = [ps2.tile([128, W], F32, tag="ps", name=f"c2ps{i}")
                    for i in range(NT)]
            for g, mts in enumerate(((0, 1, 2, 3), (4, 5, 6, 7))):
                for tap in range(3):
                    for a in range(NT // 2):
                        for mt in mts:
                            nc.tensor.matmul(
                                pss2[mt][:],
                                c2b[:, tap, a, :, mt * 128:(mt + 1) * 128],
                                y1[:, 2 * a:2 * a + 2, tap:tap + W],
                                start=(tap == 0 and a == 0),
                                stop=(tap == 2 and a == NT // 2 - 1),
                                perf_mode=DR)
                for mt in mts:
                    nc.scalar.activation(out=y2own[:, mt, :],
                                         in_=pss2[mt][:], func=AF.Relu,
                                         bias=b2[:, mt:mt + 1],
                                         scale=1.0 / 32.0)
                    nc.gpsimd.dma_start(out=y2i[g][:, mt - 4 * g, :],
                                        in_=y2own[:, mt, :])
                nc.gpsimd.collective_compute(
                    "AllGather", ALU.bypass, replica_groups=RG,
                    ins=[y2i[g][:].opt()], outs=[y2o[g][:].opt()])
            # chunk g slot s = global mid-tiles 4g..4g+4 x L-half s
            for g in range(2):
                for s in range(2):
                    nc.gpsimd.dma_start(
                        out=y2g[:, 4 * g:4 * g + 4, s * W:(s + 1) * W],
                        in_=y2o[g][s])

        # ------- conv3 (1x1) + bn3 + residual + LOCAL stats ----------
        # output channels are this core's rolled 512 over full L, so the
        # instance-norm stats need no collective at all. The contraction
        # pre-starts on mid-tiles 0-3 (first y2 chunk) for all four output
        # tiles while the second chunk is still in flight.
        with tc.tile_pool(name="c3_ps", bufs=4, space="PSUM") as ps3, \
             tc.tile_pool(name="fin_sb", bufs=1) as fsb:
            st = pm.tile([128, 8], F32, tag="st")
            yp = fsb.tile([128, 4, L // 2], F32, tag="yp")
            pscs = [ps3.tile([128, 2, W], F32, tag="ps", name=f"c3ps{ct}")
                    for ct in range(4)]
            for ct in range(4):
                for kt in range(4):
                    for lh in range(2):
                        nc.tensor.matmul(
                            pscs[ct][:, lh, :],
                            c3[:, kt, ct * 128:(ct + 1) * 128],
                            y2g[:, kt, lh * W:(lh + 1) * W],
                            start=(kt == 0), stop=False)
            for ct in range(4):
                for kt in range(4, NT):
                    for lh in range(2):
                        nc.tensor.matmul(
                            pscs[ct][:, lh, :],
                            c3[:, kt, ct * 128:(ct + 1) * 128],
                            y2g[:, kt, lh * W:(lh + 1) * W],
                            start=False, stop=(kt == NT - 1))
                y_s = fsb.tile([128, L], F32, tag="ysc", bufs=2)
                nc.vector.scalar_tensor_tensor(
                    out=y_s[:].rearrange("p (a b) -> p a b", a=2),
                    in0=pscs[ct][:], scalar=b3[:, ct:ct + 1],
                    in1=h_full[:, ct, :].rearrange("p (a b) -> p a b", a=2),
                    op0=ALU.add, op1=ALU.add)
                nc.vector.reduce_sum(out=st[:, 2 * ct:2 * ct + 1],
                                     in_=y_s[:], axis=AX.X)
                scr = fsb.tile([128, L], F32, tag="scr", bufs=2)
                nc.scalar.activation(out=scr[:], in_=y_s[:],
                                     func=AF.Square, scale=1.0 / 32.0,
                                     accum_out=st[:, 2 * ct + 1:2 * ct + 2])
                yv = y_s[:].rearrange("p (l t) -> p l t", t=2)
                nc.vector.tensor_max(out=yp[:, ct, :].unsqueeze(2),
                                     in0=yv[:, :, 0:1], in1=yv[:, :, 1:2])

            eps_sb = pm.tile([128, 1], F32, tag="eps_sb")
            nc.vector.memset(eps_sb[:], EPS)
            mean = pm.tile([128, 4], F32, tag="mean")
            ms = pm.tile([128, 4], F32, tag="ms")
            rstd = pm.tile([128, 4], F32, tag="rstd")
            shift = pm.tile([128, 4], F32, tag="shift")
            yo = fsb.tile([128, 4, L // 2], F32, tag="yo")

            def finalize(lo, hi):
                # stats chunk [lo,hi): normalize+relu+store per tile
                nc.vector.tensor_scalar_mul(
                    out=mean[:, lo:hi], in0=st[:, 2 * lo:2 * hi:2],
                    scalar1=1.0 / L)
                nc.vector.tensor_mul(out=shift[:, lo:hi], in0=mean[:, lo:hi],
                                     in1=mean[:, lo:hi])
                nc.vector.tensor_sub(out=ms[:, lo:hi],
                                     in0=st[:, 2 * lo + 1:2 * hi:2],
                                     in1=shift[:, lo:hi])
                nc.scalar.activation(out=ms[:, lo:hi], in_=ms[:, lo:hi],
                                     func=AF.Sqrt, bias=eps_sb[:], scale=1.0)
                nc.vector.reciprocal_approx_fast(out=rstd[:, lo:hi],
                                                 in_=ms[:, lo:hi])
                nc.vector.tensor_scalar(out=shift[:, lo:hi],
                                        in0=mean[:, lo:hi],
                                        scalar1=-1.0, scalar2=0.0,
                                        op0=ALU.mult, op1=ALU.add)
                nc.vector.tensor_mul(out=shift[:, lo:hi], in0=shift[:, lo:hi],
                                     in1=rstd[:, lo:hi])
                for ct in range(lo, hi):
                    nc.scalar.activation(
                        out=yo[:, ct, :], in_=yp[:, ct, :], func=AF.Relu,
                        scale=rstd[:, ct:ct + 1], bias=shift[:, ct:ct + 1])
                    nc.scalar.dma_start(
                        out=out_d[:].rearrange(
                            "(a p) l -> p a l", p=128)[:, ct, :],
                        in_=yo[:, ct, :])

            finalize(0, 2)
            finalize(2, 4)
        stB.close()

    nc.compile()
    return nc


_NC = None


def _get_nc():
    global _NC
    if _NC is None:
        _NC = _build()
    return _NC


def _prep_inputs(inputs):
    f = lambda k: np.asarray(inputs[k], dtype=np.float32)
    bf = lambda a: np.ascontiguousarray(a.astype(ml_dtypes.bfloat16))
    x = f("x")

    s0 = f("norm_g") / np.sqrt(f("norm_v") + EPS)
    t0 = f("norm_b") - f("norm_m") * s0

    # q/k/v/o weights x32 in fp8 (0.02-scale weights would sit below the
    # e4m3 min normal); the scale unwinds in the exp ACTIVATE (1/32768)
    # and the residual add (1/1024)
    q8w = lambda a: np.ascontiguousarray(
        (np.asarray(a, np.float32) * 32.0).astype(E4))

    s1 = f("bn1_g") / np.sqrt(f("bn1_v") + EPS)
    b1 = s1 * (f("cb1") - f("bn1_m")) + f("bn1_b")
    l1T_f = (s1[:, None] * f("cw1")[:, :, 0]).T

    s2 = f("bn2_g") / np.sqrt(f("bn2_v") + EPS)
    b2 = s2 * (f("cb2") - f("bn2_m")) + f("bn2_b")
    cw2 = f("cw2")
    l2T_f = np.stack([(s2[:, None] * cw2[:, :, k]).T for k in range(3)],
                     axis=0)

    s3 = f("bn3_g") / np.sqrt(f("bn3_v") + EPS)
    b3 = s3 * (f("cb3") - f("bn3_m")) + f("bn3_b")
    l3T_f = (s3[:, None] * f("cw3")[:, :, 0]).T

    selm = np.zeros((2, 128), np.float32)
    selm[0, :DH] = 1.0
    selm[1, DH:] = 1.0

    def cols(v, nt=8):  # (nt*128,) -> (128, nt): chan c = col*128 + part
        return np.ascontiguousarray(v.reshape(nt, 128).T.astype(np.float32))

    # per-half variants: the channel space is ROLLED by half*512 per core
    # (my channels = rolled tiles 0-3) so the SPMD program stays uniform;
    # everything that touches the channel axis rolls with it. The mid
    # space of conv2's output / conv3's contraction is sharded by slicing
    # the weight columns (l2T) while y2g is reassembled in GLOBAL mid
    # order by the AllGather, so l3T rows stay global.
    hv = []
    for half in range(2):
        R = half * 512
        rl = lambda a, ax: np.roll(a, -R, axis=ax)
        vecs = np.zeros((128, 39), np.float32)
        vecs[:, 0:8] = cols(rl(s0, 0))
        vecs[:, 8:16] = cols(rl(t0, 0))
        vecs[:, 16:24] = cols(b1)
        vecs[:, 24:32] = cols(b2)
        vecs[:, 32:36] = cols(b3[R:R + 512], 4)
        vecs[:, 36] = 1.0 if half == 0 else 0.0   # mA
        vecs[:, 37] = 0.0 if half == 0 else 1.0   # mB
        vecs[:, 38] = 1.0 / 1024.0
        hv.append({
            "wqT": q8w(rl(f("wq").T, 0)),
            "wkT": q8w(rl(f("wk").T, 0)),
            "wvT": q8w(rl(f("wv").T, 0)),
            "woT": q8w(rl(f("wo").T, 1)),
            "l1T": bf(rl(l1T_f, 0)),
            "l2T": np.ascontiguousarray((l2T_f * 32.0).astype(E4)),
            "l3T": bf(l3T_f[:, R:R + 512]),
            "vecs": vecs, "selm": selm,
        })

    in_maps = []
    for core in range(8):
        n, half = core // 2, core % 2
        xc = x[n] if half == 0 else np.roll(x[n], -W, axis=1)
        xc = np.roll(xc, -half * 512, axis=0)
        in_maps.append(dict(hv[half], x=bf(xc)))
    return in_maps


def kernel(**inputs):
    global LAST_RESULTS
    nc = _get_nc()
    in_maps = _prep_inputs(inputs)
    res = bass_utils.run_bass_kernel_spmd(
        nc, in_maps, core_ids=list(range(8)), trace=TRACE)
    LAST_RESULTS = res
    out = np.empty((N_BATCH, C, L // 2), np.float32)
    for core in range(8):
        n, half = core // 2, core % 2
        out[n][half * 512:(half + 1) * 512, :] = res.results[core]["out"]
    return out



# revision 23
# speedup vs baseline: 1.0947x; 1.0947x over previous
"""Trainium2 Bass kernel for nn_ExampleEncoderLayer (dense transformer block).

Sharding: hybrid batch x sequence over 8 cores = 4 batches x 2 L-halves.
Per core (batch n, half): BN(x) -> h0 (full L, for K/V); Q + attention for
its 512-column window (inputs pre-rolled on host so the window is always
local columns [0,512)); out-projection + residual. The IbnNet conv stack
then switches to channel sharding: conv1 stays L-local (all mids), a pair
AllGather reassembles y1 over the full global L, conv2 computes this
core's 512 mid channels (weight columns sliced host-side) over full L,
a second AllGather rebuilds y2 in global mid order, and conv3 computes
this core's 512 OUTPUT channels (channel space rolled per core host-side
so the SPMD program is uniform) over full L -- which makes the
instance-norm statistics core-local: no stats AllReduce, no conv2 halo
exchange. The attention residual h is pair-exchanged (AllGather + mask
blend) to cover the remote L-half. All collectives are chunked so they
overlap the neighboring matmul phases.

v2: weights/activations in bf16 (same PE rate as f32r, half the HBM/SBUF
traffic); K/Q/V/exp attention operands in fp8e4 (raw exp(s) is O(1) so the
range fits; the whole attention branch contributes ~1.3% of the residual
so fp8's ~4% relative noise lands ~1e-4 on the output, far under the 2e-2
budget). The kernel front is software-pipelined per attention PAIR: the
softmax exp stream on the Activation engine (~75us, the real bottleneck of
the attention phase) starts ~17us in and hides under the K/Q/V/AV matmuls
instead of serializing after them. GpSimd drains the K/Q/V psums so the
DVE queue stays on the oT/den/residual path. The softmax 1/sqrt(d_model)
is applied as the exp ACTIVATE's scale constant so q/k stay at full scale
for fp8.
"""

import sys
import os

for _p in ("/opt/trn_rl_repo", "/root/.axon_site/_ro/trn_rl_repo"):
    if os.path.isdir(_p) and _p not in sys.path:
        sys.path.insert(0, _p)

import numpy as np
import ml_dtypes

E4 = ml_dtypes.float8_e4m3fn

import concourse.tile as tile
from concourse import bacc, mybir
from concourse import bass_utils

F32 = mybir.dt.float32
F32R = mybir.dt.float32r
BF16 = mybir.dt.bfloat16
FP8 = mybir.dt.float8e4
AF = mybir.ActivationFunctionType
ALU = mybir.AluOpType
AX = mybir.AxisListType
DR = mybir.MatmulPerfMode.DoubleRow

C = 1024      # d_model / channels / mid_channels
L = 1024      # sequence length
N_BATCH = 4
W = 512       # per-core L window
NT = C // 128  # 8 channel tiles
HEADS = 16
DH = 64
PAIRS = 8     # head pairs (2 heads = 128 partitions)
EPS = 1e-5
RG = [[0, 1], [2, 3], [4, 5], [6, 7]]  # core pairs sharing a batch

TRACE = False
LAST_RESULTS = None


def _build():
    from contextlib import ExitStack

    nc = bacc.Bacc("TRN2", target_bir_lowering=False, debug=False, num_devices=8)

    x_d = nc.dram_tensor("x", [C, L], BF16, kind="ExternalInput").ap()
    wqT_d = nc.dram_tensor("wqT", [C, C], FP8, kind="ExternalInput").ap()
    wkT_d = nc.dram_tensor("wkT", [C, C], FP8, kind="ExternalInput").ap()
    wvT_d = nc.dram_tensor("wvT", [C, C], FP8, kind="ExternalInput").ap()
    woT_d = nc.dram_tensor("woT", [C, C], FP8, kind="ExternalInput").ap()
    l1T_d = nc.dram_tensor("l1T", [C, C], BF16, kind="ExternalInput").ap()
    l2T_d = nc.dram_tensor("l2T", [3, C, C], FP8, kind="ExternalInput").ap()
    # conv3 weights hold only this core's 512 output channels
    l3T_d = nc.dram_tensor("l3T", [C, C // 2], BF16, kind="ExternalInput").ap()
    # packed per-channel columns: s0 t0 b1 b2 (8 each) b3 (4) mA mB cinv
    vecs_d = nc.dram_tensor("vecs", [128, 39], F32, kind="ExternalInput").ap()
    # 2x128 selector for the denominator broadcast matmul:
    # row 0 = [1]*64+[0]*64, row 1 = [0]*64+[1]*64
    selm_d = nc.dram_tensor("selm", [2, 128], F32R, kind="ExternalInput").ap()
    # this core's 512 (rolled) channels x full pooled length
    out_d = nc.dram_tensor("out", [C // 2, L // 2], F32, kind="ExternalOutput").ap()

    with tile.TileContext(nc) as tc:
      with (
        tc.tile_pool(name="pmisc", bufs=1) as pm,
        tc.tile_pool(name="pB", bufs=1) as pB,
        tc.tile_pool(name="dram", bufs=1, space="DRAM") as dp,
      ):
        vecs = pm.tile([128, 39], F32, tag="vecs")
        nc.scalar.dma_start(out=vecs[:], in_=vecs_d)
        s0 = vecs[:, 0:8]
        t0 = vecs[:, 8:16]
        b1 = vecs[:, 16:24]
        b2 = vecs[:, 24:32]
        b3 = vecs[:, 32:36]
        mA = vecs[:, 36:37]
        mB = vecs[:, 37:38]
        cinv = vecs[:, 38:39]

        def wdma(**kw):
            # all weight streams on the sync HWDGE queue: scalar is reserved
            # for ACT(exp) + x staging, gpsimd for psum drains + collectives
            nc.sync.dma_start(**kw)

        ones_f = pm.tile([128, 2], F32, tag="ones_f")
        nc.vector.memset(ones_f[:], 1.0)
        selm = pm.tile([2, 128], F32R, tag="selm")
        nc.sync.dma_start(out=selm[:], in_=selm_d)

        # conv-phase buffers (persist past the attention pool)
        h = [pB.tile([128, W], BF16, tag=f"h{i}", name=f"h{i}")
             for i in range(NT)]
        c1 = pB.tile([128, NT, C], BF16, tag="c1band")

        stA = ExitStack()
        pA = stA.enter_context(tc.tile_pool(name="pA", bufs=1))

        # h0 split: window half (lives through the residual) and far half
        # (only needed for K/V)
        h0a = pA.tile([128, NT, W], BF16, tag="h0a")
        h8a = pA.tile([128, NT, W], FP8, tag="h8a")
        h8b = pA.tile([128, NT, L - W], FP8, tag="h8b")
        v_sb = pA.tile([128, NT, HEADS, DH + 1], FP8, tag="v_sb")
        nc.vector.tensor_copy(
            out=v_sb[:, :, :, DH:DH + 1],
            in_=ones_f[:, 0:1].broadcast_to((128, NT * HEADS)).rearrange(
                "p (a h) -> p a h", a=NT).unsqueeze(3))
        kT = [pA.tile([128, L], FP8, tag=f"kT{i}", name=f"kT{i}")
              for i in range(PAIRS)]
        # Q^T raw pair layout: head-A dims on partitions 0:64, head-B on
        # 64:128. Scores run as K=64 row-TILED matmul pairs (tile_position
        # (0,0)/(64,0) auto-derived from the slices) so both heads' score
        # tiles stream concurrently through the PE array.
        qT = [pA.tile([128, W], FP8, tag=f"qT{i}", name=f"qT{i}")
              for i in range(PAIRS)]
        oT = [pA.tile([128, W], BF16, tag=f"oT{i}", name=f"oT{i}")
              for i in range(PAIRS)]
        o8 = pA.tile([128, PAIRS, W], FP8, tag="o8")

        def h8key(ct, khalf):
            # key-half view of BN(x), fp8: 0 -> window half, 1 -> far half
            return h8a[:, ct, :] if khalf == 0 else h8b[:, ct, :]

        def h8pair(a, khalf, kcols=None):
            t = h8a if khalf == 0 else h8b
            v = t[:, 2 * a:2 * a + 2, :]
            return v if kcols is None else v[:, :, kcols[0]:kcols[1]]

        # --- attention bookkeeping shared by the emission helpers ---
        # per-PAIR exp tiles [128, head, kt, W] so one exp ACT covers both
        # heads of a key tile and the AV DoubleRow rhs [128, 2, W] slices out
        expT = [pA.tile([128, 2, NT, W], FP8, tag=f"expT{i}",
                        name=f"expT{i}") for i in range(PAIRS)]
        den2s = [None] * PAIRS
        dden = dp.tile([HEADS, W], F32, tag="dden")

        # PSUM pools, LIFO-ordered. Budget 8 banks of 2KB/partition:
        #   spsq (scores, 2x2 banks double-buffered) resident through
        #   attention + psA (K/Q, 3) during the K/Q phase, psV (3) during V,
        #   psO (AV, 2) + dpsn (den bcast, 1) during AV; all closed before
        #   the out-projection opens psW (4).
        stS = ExitStack()
        spsq = stS.enter_context(tc.tile_pool(name="sc_ps", bufs=2, space="PSUM"))
        psO = None  # AV psum pool: opened after the merged K/Q/V phase

        # ---------------- emission helpers ----------------
        sunits = []   # pending (pr, kt) score+exp units

        def stage_scores(pr):
            for kt in range(NT):
                sunits.append((pr, kt))

        def emit_sunit():
            if not sunits:
                return
            pr, kt = sunits.pop(0)
            sq = spsq.tile([128, 2, W], F32, tag="sq", name="sq")
            # row-tiled K=64 pair: head A in array rows 0:63, head B in
            # 64:127, streaming concurrently (tile_position auto-derives
            # from the slices' base partitions)
            nc.tensor.matmul(sq[:, 0, :], kT[pr][0:DH, kt * 128:(kt + 1) * 128],
                             qT[pr][0:DH, :])
            nc.tensor.matmul(sq[:, 1, :], kT[pr][DH:128, kt * 128:(kt + 1) * 128],
                             qT[pr][DH:128, :])
            # one ACT call per 2 banks (the 352-cycle ACTIVATE overhead is
            # per instruction); the softmax /sqrt(d_model) rides the free
            # affine scale
            nc.scalar.activation(out=expT[pr][:, :, kt, :],
                                 in_=sq[:], func=AF.Exp, scale=1.0 / 32768.0)

        def emit_sunits(n):
            for _ in range(n):
                emit_sunit()

        def emit_av(head):
            # AV for one head (DoubleRow: two key tiles per matmul); stash
            # UNNORMALIZED o^T; denominator row (the ones-column of v_sb)
            # goes to partitions 0/1 of den2f via a DRAM bounce (a partition
            # move the DVE cannot do)
            pr, hh = divmod(head, 2)
            ops = psO.tile([DH + 1, W], F32, tag="po", name="avps")
            for g in range(NT // 2):
                nc.tensor.matmul(
                    ops[:], v_sb[:, 2 * g:2 * g + 2, head, :],
                    expT[pr][:, hh, 2 * g:2 * g + 2, :],
                    start=(g == 0), stop=(g == NT // 2 - 1),
                    perf_mode=DR)
            lo, hi = hh * DH, (hh + 1) * DH
            nc.vector.tensor_copy(out=oT[pr][lo:hi, :], in_=ops[0:DH, :])
            denst = pm.tile([128, W], F32, tag="denst", bufs=2)
            nc.vector.tensor_copy(out=denst[DH:DH + 1, :],
                                  in_=ops[DH:DH + 1, :])
            nc.gpsimd.dma_start(out=dden[head:head + 1, :],
                                in_=denst[DH:DH + 1, :])
            if hh == 1:
                den2f = pm.tile([2, W], F32, tag="den2f", bufs=2)
                nc.gpsimd.dma_start(out=den2f[:],
                                    in_=dden[2 * pr:2 * pr + 2, :])
                den2r = pm.tile([2, W], F32, tag="den2r", bufs=2)
                nc.vector.reciprocal_approx_fast(out=den2r[:], in_=den2f[:])
                den2 = pm.tile([2, W], F32R, tag="den2", bufs=3)
                nc.vector.tensor_copy(out=den2[:], in_=den2r[:])
                den2s[pr] = den2

        avq = list(range(HEADS))  # heads whose AV is still pending

        def emit_avs(n):
            for _ in range(n):
                if avq:
                    emit_av(avq.pop(0))

        # ---------------- BN + per-pair K/Q, pipelined -------------------
        with tc.tile_pool(name="wband", bufs=4) as wb, \
             tc.tile_pool(name="wbandv", bufs=2) as wbv, \
             tc.tile_pool(name="xstage", bufs=3) as xsp:
          with tc.tile_pool(name="kq_ps", bufs=1, space="PSUM") as psA:
            # warm the PE clock (HAM) with throwaway matmuls while the x/
            # weight DMAs are in flight; ~3.4us of PE activity flips the
            # clock gate to 8/8 before the real work arrives
            wps = psA.tile([128, 2, W], F32, tag="kq", bufs=1)
            for i in range(36):
                nc.tensor.matmul(wps[:, 0, 0:128], selm[:], selm[:, 0:128],
                                 start=True, stop=True)

            # resident wk/wq; low halves first so pair 0 starts ASAP,
            # x tiles next, high halves after (contiguous half-DMAs hit
            # HBM line rate)
            kqK = wb.tile([128, NT // 2, 2, C], FP8, tag="kqK", bufs=1)
            kqQ = wb.tile([128, NT // 2, 2, C], FP8, tag="kqQ", bufs=1)
            x_sbs = []

            def stage_x(ct):
                x_sb = xsp.tile([128, L], BF16, tag="xs", name=f"xs{ct}")
                nc.sync.dma_start(out=x_sb[:],
                                  in_=x_d[ct * 128:(ct + 1) * 128, :])
                x_sbs.append(x_sb)

            # first two x tiles ahead of the weights: the BN chain that
            # gates the first kT drain (and so the whole exp stream) starts
            # as early as possible
            stage_x(0)
            stage_x(1)
            wdma(out=kqK[:, :, :, 0:512],
                 in_=wkT_d[:, 0:512].rearrange(
                     "(a two p) c -> p a two c", two=2, p=128))
            wdma(out=kqQ[:, :, :, 0:512],
                 in_=wqT_d[:, 0:512].rearrange(
                     "(a two p) c -> p a two c", two=2, p=128))
            for ct in range(2, NT):
                stage_x(ct)
            wdma(out=kqK[:, :, :, 512:1024],
                 in_=wkT_d[:, 512:1024].rearrange(
                     "(a two p) c -> p a two c", two=2, p=128))
            wdma(out=kqQ[:, :, :, 512:1024],
                 in_=wqT_d[:, 512:1024].rearrange(
                     "(a two p) c -> p a two c", two=2, p=128))
            # V weight bands prefetched now; the V block runs right after
            # the K/Q pairs
            vbs = []
            for g in range(2):
                vb = wbv.tile([128, NT // 2, 2, W], FP8, tag="vband",
                              name=f"vb{g}", bufs=2)
                wdma(out=vb[:],
                     in_=wvT_d[:, g * 512:(g + 1) * 512].rearrange(
                         "(a two p) c -> p a two c", two=2, p=128))
                vbs.append(vb)

            for pr in range(PAIRS):
                kps = psA.tile([128, 2, W], F32, tag="kq", bufs=1)
                qps = psA.tile([128, W], F32, tag="q", bufs=1)
                for a in range(NT // 2):
                    if pr == 0:
                        # BN as the x tiles land (first pair only)
                        for ct in (2 * a, 2 * a + 1):
                            nc.vector.tensor_scalar(
                                out=h0a[:, ct, :], in0=x_sbs[ct][:, 0:W],
                                scalar1=s0[:, ct:ct + 1],
                                scalar2=t0[:, ct:ct + 1],
                                op0=ALU.mult, op1=ALU.add)
                            nc.vector.tensor_copy(out=h8a[:, ct, :],
                                                  in_=h0a[:, ct, :])
                            nc.vector.tensor_scalar(
                                out=h8b[:, ct, :], in0=x_sbs[ct][:, W:L],
                                scalar1=s0[:, ct:ct + 1],
                                scalar2=t0[:, ct:ct + 1],
                                op0=ALU.mult, op1=ALU.add)
                    for kh in range(2):
                        nc.tensor.matmul(
                            kps[:, kh, :],
                            kqK[:, a, :, pr * 128:(pr + 1) * 128],
                            h8pair(a, kh),
                            start=(a == 0), stop=(a == NT // 2 - 1),
                            perf_mode=DR)
                    nc.tensor.matmul(
                        qps[:], kqQ[:, a, :, pr * 128:(pr + 1) * 128],
                        h8pair(a, 0),
                        start=(a == 0), stop=(a == NT // 2 - 1),
                        perf_mode=DR)
                    # two score units of the previous pair between K/Q
                    # steps keep the exp stream fed from ~one pair in
                    if pr >= 1:
                        emit_sunits(2)
                # drains on gpsimd so the DVE stays free for the oT path;
                # the PE chews queued score units while they run
                nc.vector.tensor_copy(
                    out=kT[pr][:].rearrange("p (a w) -> p a w", a=2),
                    in_=kps[:])
                nc.vector.tensor_copy(out=qT[pr][:], in_=qps[:])
                stage_scores(pr)

          # ---------------- V projection, dense block ----------------
          # psA closed: its 3 banks host the V psums; pair 7's score units
          # interleave so the exp stream never starves while V streams.
          with tc.tile_pool(name="v_ps", bufs=1, space="PSUM") as psV:
            for g in range(2):
                for ci, chunk in enumerate(((0, 1, 2), (3, 4, 5), (6, 7))):
                    vps = psV.tile([128, 3, W], F32, tag="vps",
                                   name=f"vps{g}{ci}")
                    for a in range(NT // 2):
                        for i, kt in enumerate(chunk):
                            kh, kcol = divmod(kt * 128, W)
                            nc.tensor.matmul(
                                vps[:, i, :],
                                h8pair(a, kh, (kcol, kcol + 128)),
                                vbs[g][:, a, :, :],
                                start=(a == 0), stop=(a == NT // 2 - 1),
                                perf_mode=DR)
                    for i, kt in enumerate(chunk):
                        nc.vector.tensor_copy(
                            out=v_sb[:, kt, g * 8:(g + 1) * 8, 0:DH],
                            in_=vps[:, i, :].rearrange(
                                "p (h d) -> p h d", h=8))

        # throwaway pair-AllReduce: synchronizes the cores early so the
        # conv-phase AllGathers do not pay the accumulated trigger skew
        cc0i = dp.tile([128, 1], F32, tag="cc0i")
        cc0o = dp.tile([128, 1], F32, tag="cc0o")
        nc.sync.dma_start(out=cc0i[:], in_=ones_f[:, 0:1])
        nc.gpsimd.collective_compute(
            "AllReduce", ALU.add, replica_groups=RG,
            ins=[cc0i[:].opt()], outs=[cc0o[:].opt()])

        # outproj wo bands (fp8 pair layout), prefetched now
        obs = []
        for gi in range(2):
            ob = pA.tile([128, NT // 2, 2, W], FP8,
                         tag=f"oband{gi}", name=f"oband{gi}")
            wdma(out=ob[:],
                 in_=woT_d[:, gi * 512:(gi + 1) * 512].rearrange(
                     "(a two p) c -> p a two c", two=2, p=128))
            obs.append(ob)

        # ---------------- AV (DoubleRow) + normalization ----------------
        from contextlib import ExitStack as _ES
        stP = _ES()
        psO = stP.enter_context(tc.tile_pool(name="av_ps", bufs=2,
                                             space="PSUM"))
        dpsn = stP.enter_context(tc.tile_pool(name="dn_ps", bufs=1,
                                              space="PSUM"))

        def emit_norm(p):
            # broadcast both heads' 1/den with one K=2 matmul, then
            # scale o^T in place
            dps = dpsn.tile([128, W], F32, tag="dn", name="dnps")
            nc.tensor.matmul(dps[:], selm[:], den2s[p][:])
            nc.vector.tensor_mul(out=o8[:, p, :], in0=oT[p][:],
                                 in1=dps[:])

        # prefetch conv1 weights before the AV/outproj phase (the sync
        # queue is otherwise idle here and conv1 starts right after)
        wdma(out=c1[:], in_=l1T_d[:].rearrange("(a p) c -> p a c", p=128))

        for p in range(PAIRS):
            emit_avs(2)
            # norm of the pair whose den-reciprocal chain (DRAM bounce +
            # DVE) has certainly completed; lag 2 pairs
            if p >= 2:
                emit_norm(p - 2)
            if p == 5:
                # pair 7's score units, paced by the exp stream's sq-bank
                # releases (the stream has nearly drained by now)
                emit_sunits(8)
        emit_norm(PAIRS - 2)
        emit_norm(PAIRS - 1)

        stP.close()  # AV + den psum pools close
        stS.close()  # scores psum pool closes

        # second throwaway pair-sync: absorbs the core skew accumulated
        # across the attention phase so the conv-phase AllGathers process
        # at their ~5us floor instead of paying the slowest core's lag
        cc1i = dp.tile([128, 1], F32, tag="cc1i")
        cc1o = dp.tile([128, 1], F32, tag="cc1o")
        nc.sync.dma_start(out=cc1i[:], in_=ones_f[:, 0:1])
        nc.gpsimd.collective_compute(
            "AllReduce", ALU.add, replica_groups=RG,
            ins=[cc1i[:].opt()], outs=[cc1o[:].opt()])

        # ---------------- out-projection ----------------
        # h tiles 4:8 first so their pair-exchange staging can start while
        # tiles 0:4 are still accumulating
        # hx payload: peer-channel h tiles + 4 trailing cols per tile that
        # carry the conv1 window-edge outputs (the conv2 halo), so ONE
        # AllGather serves both exchanges
        hxi = dp.tile([128, 4, W + 4], BF16, tag="hxi")
        hxo = dp.tile([2, 128, 4, W + 4], BF16, tag="hxo")
        with tc.tile_pool(name="wo_ps", bufs=2, space="PSUM") as psW:

            def op_group(cts, kts, pss, first, last):
                gi = cts[0] // 4
                for kt in kts:
                    for i, ct in enumerate(cts):
                        nc.tensor.matmul(
                            pss[i][:],
                            obs[gi][:, kt // 2, kt % 2,
                                    (ct % 4) * 128:(ct % 4 + 1) * 128],
                            o8[:, kt, :],
                            start=(kt == first), stop=(kt == last))

            for gX in ((4, 5), (6, 7), (0, 1), (2, 3)):
                wopX = [psW.tile([128, W], F32, tag="wo",
                                 name=f"wop{gX[0]}_{i}") for i in range(2)]
                op_group(gX, tuple(range(NT)), wopX, 0, NT - 1)
                for i, ct in enumerate(gX):
                    nc.vector.scalar_tensor_tensor(
                        out=h[ct][:], in0=wopX[i][:], scalar=cinv,
                        in1=h0a[:, ct, :], op0=ALU.mult, op1=ALU.add)
                if gX == (6, 7):
                    for i in range(4):
                        nc.gpsimd.dma_start(out=hxi[:, i, 0:W],
                                            in_=h[4 + i][:])

        # attention-phase SBUF is no longer needed; conv buffers take its
        # place in pools opened only now (pools close LIFO, hence the split).
        stA.close()
        stB = ExitStack()
        pC = stB.enter_context(tc.tile_pool(name="pC", bufs=1))
        h_full = pC.tile([128, 4, L], BF16, tag="h_full")
        with (
            tc.tile_pool(name="c1_ps", bufs=8, space="PSUM") as ps1,
        ):
            # resident conv2/conv3 weights, streamed during conv1
            c2b = pC.tile([128, 3, 4, 2, C], FP8, tag="c2b")
            for tap in range(3):
                nc.sync.dma_start(
                    out=c2b[:, tap, :, :, :],
                    in_=l2T_d[tap].rearrange("(a two p) c -> p a two c",
                                             two=2, p=128))
            c3 = pC.tile([128, NT, W], BF16, tag="c3b")
            nc.sync.dma_start(
                out=c3[:], in_=l3T_d[:].rearrange("(a p) c -> p a c", p=128))

            # ---------------- conv1 (1x1) + bn1 + relu ----------------
            # L-local: all 1024 mids x my 512 cols. y1 col j+1 = my col j;
            # cols 0/513 are the cross-core halo columns (row stride 528
            # keeps the DoubleRow Ko-step 16-aligned).
            y1 = pC.tile([128, NT, 528], FP8, tag="y1")
            c1bands = [c1[:, kt, :] for kt in range(NT)]
            # boundary pre-chain first: the two window-edge output columns
            # feed the tiny edge AllGather that replaces the halo exchange
            bps = [ps1.tile([128, 2], F32, tag="ps", name=f"bps{i}")
                   for i in range(NT)]
            for kt in range(NT):
                for mt in range(NT):
                    nc.tensor.matmul(
                        bps[mt][:], c1bands[kt][:, mt * 128:(mt + 1) * 128],
                        h[kt][:, 0:W:W - 1],
                        start=(kt == 0), stop=(kt == NT - 1))
            bc = pm.tile([128, NT, 2], F32, tag="bc")
            for mt in range(NT):
                nc.vector.tensor_scalar(
                    out=bc[:, mt, :], in0=bps[mt][:],
                    scalar1=b1[:, mt:mt + 1], scalar2=0.0,
                    op0=ALU.add, op1=ALU.max)
            bc16 = pm.tile([128, 16], BF16, tag="bc16")
            nc.vector.tensor_copy(out=bc16[:],
                                  in_=bc[:].rearrange("p a b -> p (a b)"))
            for i in range(4):
                nc.gpsimd.dma_start(out=hxi[:, i, W:W + 4],
                                    in_=bc16[:, 4 * i:4 * i + 4])
            nc.gpsimd.collective_compute(
                "AllGather", ALU.bypass, replica_groups=RG,
                ins=[hxi[:].opt()], outs=[hxo[:].opt()])
            # edge readback + halo blend: left halo col = peer edge col 511
            # (zero at the global left edge via mB), right halo = peer col 0
            exs = pm.tile([128, 2, 16], BF16, tag="exs")
            for s in range(2):
                for i in range(4):
                    nc.gpsimd.dma_start(
                        out=exs[:, s, 4 * i:4 * i + 4],
                        in_=hxo[s][:, i, W:W + 4])
            nc.vector.tensor_scalar_mul(
                out=y1[:, :, 0:1],
                in0=exs[:, 0, 1:16:2].unsqueeze(2), scalar1=mB)
            nc.vector.tensor_scalar_mul(
                out=y1[:, :, W + 1:W + 2],
                in0=exs[:, 1, 0:16:2].unsqueeze(2), scalar1=mA)

            # h_full blend: remote-L half comes from the peer's hx slot; the
            # mA/mB masks pick the right slot per core (emitted now, runs as
            # soon as the hx AllGather lands; DVE is idle through conv1/2)
            hxs = pC.tile([128, 2, 4, W], BF16, tag="hxs")
            for s in range(2):
                nc.gpsimd.dma_start(out=hxs[:, s, :, :],
                                    in_=hxo[s][:, :, 0:W])
            hblt = pC.tile([128, 4, W], BF16, tag="hblt")
            for ct in range(4):
                nc.vector.tensor_scalar_mul(out=hblt[:, ct, :],
                                            in0=hxs[:, 0, ct, :], scalar1=mB)
                nc.vector.scalar_tensor_tensor(
                    out=h_full[:, ct, 0:W], in0=h[ct][:], scalar=mA,
                    in1=hblt[:, ct, :], op0=ALU.mult, op1=ALU.add)
                nc.vector.tensor_scalar_mul(out=hblt[:, ct, :],
                                            in0=hxs[:, 1, ct, :], scalar1=mA)
                nc.vector.scalar_tensor_tensor(
                    out=h_full[:, ct, W:L], in0=h[ct][:], scalar=mB,
                    in1=hblt[:, ct, :], op0=ALU.mult, op1=ALU.add)

            # conv1 main
            pss1 = [ps1.tile([128, W], F32, tag="ps", name=f"c1ps{i}")
                    for i in range(NT)]
            for kt in range(NT):
                for mt in range(NT):
                    nc.tensor.matmul(
                        pss1[mt][:],
                        c1bands[kt][:, mt * 128:(mt + 1) * 128],
                        h[kt][:], start=(kt == 0), stop=(kt == NT - 1))
            for mt in range(NT):
                nc.scalar.activation(out=y1[:, mt, 1:W + 1],
                                     in_=pss1[mt][:], func=AF.Relu,
                                     bias=b1[:, mt:mt + 1], scale=1.0)

        # ---------------- conv2 (k=3) + bn2 + relu, L-local ------------
        # all 1024 mids x my 512 cols; mid-tile groups 0-3 finish first so
        # their y2 AllGather chunk flies while the PE works on 4-7
        with tc.tile_pool(name="c2_ps", bufs=8, space="PSUM") as ps2:
            y2own = pC.tile([128, NT, W], FP8, tag="y2own")
            y2g = pC.tile([128, NT, L], FP8, tag="y2g")
            y2i = [dp.tile([128, 4, W], FP8, tag=f"y2i{g}", name=f"y2i{g}")
                   for g in range(2)]
            y2o = [dp.tile([2, 128, 4, W], FP8, tag=f"y2o{g}", name=f"y2o{g}")
                   for g in range(2)]
            pss2 = [ps2.tile([128, W], F32, tag="ps", name=f"c2ps{i}")
                    for i in range(NT)]
            for g, mts in enumerate(((0, 1, 2, 3), (4, 5, 6, 7))):
                for tap in range(3):
                    for a in range(NT // 2):
                        for mt in mts:
                            nc.tensor.matmul(
                                pss2[mt][:],
                                c2b[:, tap, a, :, mt * 128:(mt + 1) * 128],
                                y1[:, 2 * a:2 * a + 2, tap:tap + W],
                                start=(tap == 0 and a == 0),
                                stop=(tap == 2 and a == NT // 2 - 1),
                                perf_mode=DR)
                for mt in mts:
                    nc.scalar.activation(out=y2own[:, mt, :],
                                         in_=pss2[mt][:], func=AF.Relu,
                                         bias=b2[:, mt:mt + 1],
                                         scale=1.0 / 32.0)
                    nc.gpsimd.dma_start(out=y2i[g][:, mt - 4 * g, :],
                                        in_=y2own[:, mt, :])
                nc.gpsimd.collective_compute(
                    "AllGather", ALU.bypass, replica_groups=RG,
                    ins=[y2i[g][:].opt()], outs=[y2o[g][:].opt()])
            # chunk g slot s = global mid-tiles 4g..4g+4 x L-half s
            for g in range(2):
                for s in range(2):
                    nc.gpsimd.dma_start(
                        out=y2g[:, 4 * g:4 * g + 4, s * W:(s + 1) * W],
                        in_=y2o[g][s])

        # ------- conv3 (1x1) + bn3 + residual + LOCAL stats ----------
        # output channels are this core's rolled 512 over full L, so the
        # instance-norm stats need no collective at all. The contraction
        # pre-starts on mid-tiles 0-3 (first y2 chunk) for all four output
        # tiles while the second chunk is still in flight.
        with tc.tile_pool(name="c3_ps", bufs=4, space="PSUM") as ps3, \
             tc.tile_pool(name="fin_sb", bufs=1) as fsb:
            st = pm.tile([128, 8], F32, tag="st")
            yp = fsb.tile([128, 4, L // 2], F32, tag="yp")
            pscs = [ps3.tile([128, 2, W], F32, tag="ps", name=f"c3ps{ct}")
                    for ct in range(4)]
            for ct in range(4):
                for kt in range(4):
                    for lh in range(2):
                        nc.tensor.matmul(
                            pscs[ct][:, lh, :],
                            c3[:, kt, ct * 128:(ct + 1) * 128],
                            y2g[:, kt, lh * W:(lh + 1) * W],
                            start=(kt == 0), stop=False)
            for ct in range(4):
                for kt in range(4, NT):
                    for lh in range(2):
                        nc.tensor.matmul(
                            pscs[ct][:, lh, :],
                            c3[:, kt, ct * 128:(ct + 1) * 128],
                            y2g[:, kt, lh * W:(lh + 1) * W],
                            start=False, stop=(kt == NT - 1))
                y_s = fsb.tile([128, L], F32, tag="ysc", bufs=2)
                nc.vector.scalar_tensor_tensor(
                    out=y_s[:].rearrange("p (a b) -> p a b", a=2),
                    in0=pscs[ct][:], scalar=b3[:, ct:ct + 1],
                    in1=h_full[:, ct, :].rearrange("p (a b) -> p a b", a=2),
                    op0=ALU.add, op1=ALU.add)
                nc.vector.reduce_sum(out=st[:, 2 * ct:2 * ct + 1],
                                     in_=y_s[:], axis=AX.X)
                scr = fsb.tile([128, L], F32, tag="scr", bufs=2)
                nc.scalar.activation(out=scr[:], in_=y_s[:],
                                     func=AF.Square, scale=1.0 / 32.0,
                                     accum_out=st[:, 2 * ct + 1:2 * ct + 2])
                yv = y_s[:].rearrange("p (l t) -> p l t", t=2)
                nc.vector.tensor_max(out=yp[:, ct, :].unsqueeze(2),
                                     in0=yv[:, :, 0:1], in1=yv[:, :, 1:2])

            eps_sb = pm.tile([128, 1], F32, tag="eps_sb")
            nc.vector.memset(eps_sb[:], EPS)
            mean = pm.tile([128, 4], F32, tag="mean")
            ms = pm.tile([128, 4], F32, tag="ms")
            rstd = pm.tile([128, 4], F32, tag="rstd")
            shift = pm.tile([128, 4], F32, tag="shift")
            yo = fsb.tile([128, 4, L // 2], F32, tag="yo")

            def finalize(lo, hi):
                # stats chunk [lo,hi): normalize+relu+store per tile
                nc.vector.tensor_scalar_mul(
                    out=mean[:, lo:hi], in0=st[:, 2 * lo:2 * hi:2],
                    scalar1=1.0 / L)
                nc.vector.tensor_mul(out=shift[:, lo:hi], in0=mean[:, lo:hi],
                                     in1=mean[:, lo:hi])
                nc.vector.tensor_sub(out=ms[:, lo:hi],
                                     in0=st[:, 2 * lo + 1:2 * hi:2],
                                     in1=shift[:, lo:hi])
                nc.scalar.activation(out=ms[:, lo:hi], in_=ms[:, lo:hi],
                                     func=AF.Sqrt, bias=eps_sb[:], scale=1.0)
                nc.vector.reciprocal_approx_fast(out=rstd[:, lo:hi],
                                                 in_=ms[:, lo:hi])
                nc.vector.tensor_scalar(out=shift[:, lo:hi],
                                        in0=mean[:, lo:hi],
                                        scalar1=-1.0, scalar2=0.0,
                                        op0=ALU.mult, op1=ALU.add)
                nc.vector.tensor_mul(out=shift[:, lo:hi], in0=shift[:, lo:hi],
                                     in1=rstd[:, lo:hi])
                for ct in range(lo, hi):
                    nc.scalar.activation(
                        out=yo[:, ct, :], in_=yp[:, ct, :], func=AF.Relu,
                        scale=rstd[:, ct:ct + 1], bias=shift[:, ct:ct + 1])
                    nc.scalar.dma_start(
                        out=out_d[:].rearrange(
                            "(a p) l -> p a l", p=128)[:, ct, :],
                        in_=yo[:, ct, :])

            finalize(0, 2)
            finalize(2, 4)
        stB.close()

    nc.compile()
    return nc


_NC = None


def _get_nc():
    global _NC
    if _NC is None:
        _NC = _build()
    return _NC


def _prep_inputs(inputs):
    f = lambda k: np.asarray(inputs[k], dtype=np.float32)
    bf = lambda a: np.ascontiguousarray(a.astype(ml_dtypes.bfloat16))
    x = f("x")

    s0 = f("norm_g") / np.sqrt(f("norm_v") + EPS)
    t0 = f("norm_b") - f("norm_m") * s0

    # q/k/v/o weights x32 in fp8 (0.02-scale weights would sit below the
    # e4m3 min normal); the scale unwinds in the exp ACTIVATE (1/32768)
    # and the residual add (1/1024)
    q8w = lambda a: np.ascontiguousarray(
        (np.asarray(a, np.float32) * 32.0).astype(E4))

    s1 = f("bn1_g") / np.sqrt(f("bn1_v") + EPS)
    b1 = s1 * (f("cb1") - f("bn1_m")) + f("bn1_b")
    l1T_f = (s1[:, None] * f("cw1")[:, :, 0]).T

    s2 = f("bn2_g") / np.sqrt(f("bn2_v") + EPS)
    b2 = s2 * (f("cb2") - f("bn2_m")) + f("bn2_b")
    cw2 = f("cw2")
    l2T_f = np.stack([(s2[:, None] * cw2[:, :, k]).T for k in range(3)],
                     axis=0)

    s3 = f("bn3_g") / np.sqrt(f("bn3_v") + EPS)
    b3 = s3 * (f("cb3") - f("bn3_m")) + f("bn3_b")
    l3T_f = (s3[:, None] * f("cw3")[:, :, 0]).T

    selm = np.zeros((2, 128), np.float32)
    selm[0, :DH] = 1.0
    selm[1, DH:] = 1.0

    def cols(v, nt=8):  # (nt*128,) -> (128, nt): chan c = col*128 + part
        return np.ascontiguousarray(v.reshape(nt, 128).T.astype(np.float32))

    # per-half variants: the channel space is ROLLED by half*512 per core
    # (my channels = rolled tiles 0-3) so the SPMD program stays uniform;
    # everything that touches the channel axis rolls with it. The mid
    # space of conv2's output / conv3's contraction is sharded by slicing
    # the weight columns (l2T) while y2g is reassembled in GLOBAL mid
    # order by the AllGather, so l3T rows stay global.
    hv = []
    for half in range(2):
        R = half * 512
        rl = lambda a, ax: np.roll(a, -R, axis=ax)
        vecs = np.zeros((128, 39), np.float32)
        vecs[:, 0:8] = cols(rl(s0, 0))
        vecs[:, 8:16] = cols(rl(t0, 0))
        vecs[:, 16:24] = cols(b1)
        vecs[:, 24:32] = cols(b2)
        vecs[:, 32:36] = cols(b3[R:R + 512], 4)
        vecs[:, 36] = 1.0 if half == 0 else 0.0   # mA
        vecs[:, 37] = 0.0 if half == 0 else 1.0   # mB
        vecs[:, 38] = 1.0 / 1024.0
        hv.append({
            "wqT": q8w(rl(f("wq").T, 0)),
            "wkT": q8w(rl(f("wk").T, 0)),
            "wvT": q8w(rl(f("wv").T, 0)),
            "woT": q8w(rl(f("wo").T, 1)),
            "l1T": bf(rl(l1T_f, 0)),
            "l2T": np.ascontiguousarray((l2T_f * 32.0).astype(E4)),
            "l3T": bf(l3T_f[:, R:R + 512]),
            "vecs": vecs, "selm": selm,
        })

    in_maps = []
    for core in range(8):
        n, half = core // 2, core % 2
        xc = x[n] if half == 0 else np.roll(x[n], -W, axis=1)
        xc = np.roll(xc, -half * 512, axis=0)
        in_maps.append(dict(hv[half], x=bf(xc)))
    return in_maps


def kernel(**inputs):
    global LAST_RESULTS
    nc = _get_nc()
    in_maps = _prep_inputs(inputs)
    res = bass_utils.run_bass_kernel_spmd(
        nc, in_maps, core_ids=list(range(8)), trace=TRACE)
    LAST_RESULTS = res
    out = np.empty((N_BATCH, C, L // 2), np.float32)
    for core in range(8):
        n, half = core // 2, core % 2
        out[n][half * 512:(half + 1) * 512, :] = res.results[core]["out"]
    return out

